# revision 1
# baseline (speedup 1.0000x reference)
"""Multi-head self-attention (RMSNorm + causal MHA + out-proj) on 8 TRN2 cores.

Sharding (tensor-parallel per heads x data-parallel per batch):
  core c handles batch b = c//4 and head group hg = c%4 (4 of 16 heads).
  Each core computes a PARTIAL output (its heads' slice of the out-projection
  contraction); the host sums the 4 partials per batch (the reduce inherent in
  head-split TP) and transposes back.

Device kernel (per core, everything feature-major / transposed so no on-chip
transposes are ever needed):
  RMSNorm via an all-ones 128x128 matmul (partition reduce + broadcast in one
  shot), QKV in bf16, causal flash attention computed as S^T = K^T.T @ Q^T
  (exp on ScalarE with fused 1/sqrt(d) scale, no max-subtraction -- scores are
  O(1) by construction), softmax denominator fused into the PV matmul via a
  ones column appended to V, out-projection consumes ctx^T directly.
"""

from contextlib import ExitStack

import numpy as np

import concourse.bass as bass
import concourse.tile as tile
from concourse import bacc, mybir
from concourse.bass_utils import run_bass_kernel_spmd

F32 = mybir.dt.float32
BF16 = mybir.dt.bfloat16
AF = mybir.ActivationFunctionType
P = 128
DD = 64           # head dim
T = 2048
D = 1024
NH = 4            # heads per core
N_CORES = 8
EPS = 1e-6


def _build(nc):
    KT = D // P
    TT = T // P
    TC = T // 512
    NP = NH // 2
    QK = 2 * NH * DD
    VF = NH * DD

    xT_d = nc.dram_tensor("xT", [D, T], F32, kind="ExternalInput")
    wqkT_d = nc.dram_tensor("wqkT", [D, QK], F32, kind="ExternalInput")
    wvT_d = nc.dram_tensor("wvT", [D, VF], F32, kind="ExternalInput")
    woT_d = nc.dram_tensor("woT", [VF, D], F32, kind="ExternalInput")
    wn_d = nc.dram_tensor("wn", [D, 1], F32, kind="ExternalInput")
    outT_d = nc.dram_tensor("outT", [D, T], F32, kind="ExternalOutput")

    with tile.TileContext(nc) as tc, ExitStack() as ctx:
        consts = ctx.enter_context(tc.tile_pool(name="consts", bufs=1))
        persist = ctx.enter_context(tc.tile_pool(name="persist", bufs=1))
        xstage = ctx.enter_context(tc.tile_pool(name="xstage", bufs=2))
        wstage = ctx.enter_context(tc.tile_pool(name="wstage", bufs=2))
        xsqp = ctx.enter_context(tc.tile_pool(name="xsqp", bufs=2))
        stmp = ctx.enter_context(tc.tile_pool(name="stmp", bufs=2))
        epool = ctx.enter_context(tc.tile_pool(name="epool", bufs=2))
        rlp = ctx.enter_context(tc.tile_pool(name="rlp", bufs=2))
        dramp = ctx.enter_context(tc.tile_pool(name="dramp", bufs=2, space="DRAM"))
        sps = ctx.enter_context(tc.tile_pool(name="sps", bufs=1, space="PSUM"))
        ctxps = ctx.enter_context(tc.tile_pool(name="ctxps", bufs=2, space="PSUM"))
        mmps = ctx.enter_context(tc.tile_pool(name="mmps", bufs=2, space="PSUM"))

        # ---- constants -------------------------------------------------
        ones_bf = consts.tile([P, P], BF16)
        nc.vector.memset(ones_bf[:], 1.0)
        eps_sb = consts.tile([P, 1], F32)
        nc.vector.memset(eps_sb[:], EPS)
        # causal mask in S^T orientation: keep where tq(free) >= tk(partition)
        mask_bf = consts.tile([P, P], BF16)
        nc.gpsimd.memset(mask_bf[:], 1.0)
        nc.gpsimd.affine_select(
            out=mask_bf[:], in_=mask_bf[:],
            compare_op=mybir.AluOpType.is_ge, fill=0.0, base=0,
            pattern=[[1, P]], channel_multiplier=-1,
        )
        wn_sb = consts.tile([P, KT], F32)
        for kt in range(KT):
            nc.sync.dma_start(wn_sb[:, kt : kt + 1], wn_d.ap()[kt * P : (kt + 1) * P, :])

        # ---- weights: load f32, fold norm weight, cast bf16 ------------
        wqk_bf = persist.tile([P, KT, QK], BF16)
        wv_bf = persist.tile([P, KT, VF], BF16)
        wo_bf = persist.tile([P, VF // P, D], BF16)
        for kt in range(KT):
            wst = wstage.tile([P, QK + VF], F32, tag="wst")
            nc.sync.dma_start(wst[:, :QK], wqkT_d.ap()[kt * P : (kt + 1) * P, :])
            nc.sync.dma_start(wst[:, QK:], wvT_d.ap()[kt * P : (kt + 1) * P, :])
            nc.vector.tensor_scalar(
                out=wqk_bf[:, kt, :], in0=wst[:, :QK],
                scalar1=wn_sb[:, kt : kt + 1], scalar2=None,
                op0=mybir.AluOpType.mult,
            )
            nc.vector.tensor_scalar(
                out=wv_bf[:, kt, :], in0=wst[:, QK:],
                scalar1=wn_sb[:, kt : kt + 1], scalar2=None,
                op0=mybir.AluOpType.mult,
            )
        for ct in range(VF // P):
            wost = wstage.tile([P, D], F32, tag="wost")
            nc.sync.dma_start(wost[:], woT_d.ap()[ct * P : (ct + 1) * P, :])
            nc.gpsimd.tensor_copy(out=wo_bf[:, ct, :], in_=wost[:])

        # ---- x load + RMSNorm ------------------------------------------
        xbf = persist.tile([P, KT, T], BF16)
        ms_ps = sps.tile([P, T], F32, tag="big")
        for kt in range(KT):
            xst = xstage.tile([P, T], F32)
            nc.sync.dma_start(xst[:], xT_d.ap()[kt * P : (kt + 1) * P, :])
            nc.gpsimd.tensor_copy(out=xbf[:, kt, :], in_=xst[:])
            xsq = xsqp.tile([P, T], BF16)
            nc.vector.tensor_mul(xsq[:], xbf[:, kt, :], xbf[:, kt, :])
            for c in range(TC):
                nc.tensor.matmul(
                    ms_ps[:, 512 * c : 512 * (c + 1)],
                    ones_bf[:], xsq[:, 512 * c : 512 * (c + 1)],
                    start=(kt == 0), stop=(kt == KT - 1),
                )
        rstd_bf = persist.tile([P, T], BF16)
        for c in range(TC):
            sq = stmp.tile([P, 512], F32, tag="sq")
            nc.scalar.activation(
                sq[:], ms_ps[:, 512 * c : 512 * (c + 1)],
                AF.Sqrt, bias=eps_sb[:, 0:1], scale=1.0 / D,
            )
            with nc.allow_low_precision(reason="rstd in bf16 feeds bf16 matmuls"):
                nc.vector.reciprocal(rstd_bf[:, 512 * c : 512 * (c + 1)], sq[:])
        xn = persist.tile([P, KT, T], BF16)
        for kt in range(KT):
            nc.vector.tensor_mul(xn[:, kt, :], xbf[:, kt, :], rstd_bf[:])

        # ---- QKV projections -------------------------------------------
        # Q^T/K^T per head, duplicated on both partition halves so that
        # consecutive tk-tiles' score matmuls can row-tile the PE array.
        QTd = persist.tile([P, NH, T], BF16)
        KTd = persist.tile([P, NH, T], BF16)
        for ft in range(2 * NP):
            is_k = ft // NP
            pair = ft % NP
            dst = KTd if is_k else QTd
            hA, hB = 2 * pair, 2 * pair + 1
            for c in range(TC):
                qkps = mmps.tile([P, 512], F32, tag="mm")
                for kt in range(KT):
                    nc.tensor.matmul(
                        qkps[:],
                        wqk_bf[:, kt, P * ft : P * (ft + 1)],
                        xn[:, kt, 512 * c : 512 * (c + 1)],
                        start=(kt == 0), stop=(kt == KT - 1),
                    )
                cs = slice(512 * c, 512 * (c + 1))
                nc.vector.tensor_copy(dst[0:DD, hA, cs], qkps[0:DD, :])
                nc.vector.tensor_copy(dst[DD:P, hB, cs], qkps[DD:P, :])
                nc.sync.dma_start(dst[DD:P, hA, cs], dst[0:DD, hA, cs])
                nc.sync.dma_start(dst[0:DD, hB, cs], dst[DD:P, hB, cs])

        # V token-major with a ones column per head: [tk, (v0..v63, 1) x NH]
        Vsb = persist.tile([P, TT, 65 * NH], BF16)
        for tt in range(TT):
            vps = mmps.tile([P, 512], F32, tag="mm")
            for kt in range(KT):
                nc.tensor.matmul(
                    vps[:, :VF],
                    xn[:, kt, P * tt : P * (tt + 1)],
                    wv_bf[:, kt, :],
                    start=(kt == 0), stop=(kt == KT - 1),
                )
            for h in range(NH):
                nc.vector.tensor_copy(
                    Vsb[:, tt, 65 * h : 65 * h + DD], vps[:, DD * h : DD * (h + 1)]
                )
                nc.vector.memset(Vsb[:, tt, 65 * h + DD : 65 * h + 65], 1.0)

        # ---- attention + out-projection, tq-chunk major ----------------
        ctxn = persist.tile([P, NP, T], BF16)
        for c in range(TC):
            for h in range(NH):
                ctx_ps = ctxps.tile([65, 512], F32)
                njt = 4 * (c + 1)
                for J in range(c + 1):
                    sst = sps.tile([P, 2048], F32, tag="big")
                    diag = J == c
                    for s in range(4):
                        j = 4 * J + s
                        rg = DD * (j % 2)
                        nc.tensor.matmul(
                            sst[:, 512 * s : 512 * (s + 1)],
                            KTd[rg : rg + DD, h, P * j : P * (j + 1)],
                            QTd[rg : rg + DD, h, 512 * c : 512 * (c + 1)],
                            start=True, stop=True,
                        )
                    expS = epool.tile([P, 2048], BF16)
                    nc.scalar.activation(expS[:], sst[:], AF.Exp, scale=0.125)
                    if diag:
                        for s in range(4):
                            blk = slice(640 * s, 640 * s + P)
                            nc.vector.tensor_mul(
                                expS[:, blk], expS[:, blk], mask_bf[:]
                            )
                    for s in range(4):
                        j = 4 * J + s
                        off = 128 * s if diag else 0
                        nc.tensor.matmul(
                            ctx_ps[:, off:512],
                            Vsb[:, j, 65 * h : 65 * (h + 1)],
                            expS[:, 512 * s + off : 512 * (s + 1)],
                            start=(j == 0), stop=(j == njt - 1),
                        )
                # normalize: rows 0..63 = ctx^T, row 64 = l
                rlrow = rlp.tile([DD + 1, 512], F32, tag="rlrow")
                nc.vector.reciprocal(rlrow[DD : DD + 1, :], ctx_ps[DD : DD + 1, :])
                rl_dram = dramp.tile([1, 512], F32, tag="rld")
                nc.sync.dma_start(rl_dram[:], rlrow[DD : DD + 1, :])
                rl = rlp.tile([DD, 512], F32, tag="rl")
                nc.gpsimd.dma_start(rl[:], rl_dram[0:1, :].partition_broadcast(DD))
                pair, half = h // 2, h % 2
                cs = slice(512 * c, 512 * (c + 1))
                if half == 0:
                    nc.vector.tensor_mul(ctxn[0:DD, pair, cs], ctx_ps[0:DD, :], rl[:])
                else:
                    tmpb = rlp.tile([DD, 512], BF16, tag="tmpb")
                    nc.vector.tensor_mul(tmpb[:], ctx_ps[0:DD, :], rl[:])
                    nc.sync.dma_start(ctxn[DD:P, pair, cs], tmpb[:])
            # out-projection for this chunk
            for e in range(D // P):
                ops = mmps.tile([P, 512], F32, tag="mm")
                for ct in range(NP):
                    nc.tensor.matmul(
                        ops[:],
                        wo_bf[:, ct, P * e : P * (e + 1)],
                        ctxn[:, ct, 512 * c : 512 * (c + 1)],
                        start=(ct == 0), stop=(ct == NP - 1),
                    )
                osb = stmp.tile([P, 512], F32, tag="osb")
                if e % 2 == 0:
                    nc.vector.tensor_copy(osb[:], ops[:])
                else:
                    nc.scalar.copy(osb[:], ops[:])
                nc.sync.dma_start(
                    outT_d.ap()[P * e : P * (e + 1), 512 * c : 512 * (c + 1)], osb[:]
                )


_NC_CACHE = None


def _get_nc():
    global _NC_CACHE
    if _NC_CACHE is None:
        nc = bacc.Bacc(
            "TRN2", target_bir_lowering=False, debug=False, num_devices=N_CORES
        )
        _build(nc)
        nc.compile()
        _NC_CACHE = nc
    return _NC_CACHE


def make_in_maps(x, norm_weight, qkv_w, out_w):
    x = np.asarray(x, dtype=np.float32)
    norm_weight = np.asarray(norm_weight, dtype=np.float32)
    qkv_w = np.asarray(qkv_w, dtype=np.float32)
    out_w = np.asarray(out_w, dtype=np.float32)
    wn = np.ascontiguousarray(norm_weight[:, None])
    in_maps = []
    for core in range(N_CORES):
        b, hg = core // 4, core % 4
        r = slice(256 * hg, 256 * (hg + 1))
        xT = np.ascontiguousarray(x[b].T)
        wqkT = np.ascontiguousarray(
            np.concatenate([qkv_w[r.start : r.stop], qkv_w[D + r.start : D + r.stop]], 0).T
        )
        wvT = np.ascontiguousarray(qkv_w[2 * D + r.start : 2 * D + r.stop].T)
        woT = np.ascontiguousarray(out_w[:, r].T)
        in_maps.append(
            {"xT": xT, "wqkT": wqkT, "wvT": wvT, "woT": woT, "wn": wn}
        )
    return in_maps


def gather_output(results):
    out = np.empty((2, T, D), np.float32)
    for b in range(2):
        acc = results[4 * b]["outT"].astype(np.float32).copy()
        for hg in range(1, 4):
            acc += results[4 * b + hg]["outT"]
        out[b] = acc.T
    return out


def kernel(x, norm_weight, qkv_w, out_w):
    nc = _get_nc()
    in_maps = make_in_maps(x, norm_weight, qkv_w, out_w)
    res = run_bass_kernel_spmd(nc, in_maps, core_ids=list(range(N_CORES)))
    return gather_output(res.results)


# revision 2
# speedup vs baseline: 243.7455x; 243.7455x over previous
"""Multi-head self-attention (RMSNorm + causal MHA + out-proj) on 8 TRN2 cores.

Sharding (per the tensor-parallel hint): core c handles batch b = c//4 and
head group hg = c%4 (4 of 16 heads). Each core computes a PARTIAL output (its
heads' slice of the out-projection contraction); the host sums the 4 partials
per batch — the reduce inherent in head-split TP — and transposes back.

Device kernel (per core, feature-major / transposed orientation throughout so
no on-chip transposes are ever needed):
  - RMSNorm via an all-ones 128x128 matmul (partition reduce + broadcast to
    all partitions in one shot), rstd on ScalarE sqrt + VectorE reciprocal.
  - bf16 compute on TensorE. Norm weight is folded into the projection
    weights on the host (exact), weights shipped pre-transposed bf16.
  - Causal flash attention computed transposed: S^T = K^T.T @ Q^T, with
    Q^T/K^T duplicated on both partition halves so consecutive tk tiles'
    score matmuls row-tile the PE array concurrently (K=64 each).
  - exp on ScalarE with fused 1/sqrt(d) scale; no max-subtraction (scores
    are ~N(0,1) by construction, exp is safely bounded in fp32/bf16).
  - The softmax denominator is fused into the PV matmul via a ones column
    appended to V (M=65); ctx^T and l accumulate together in PSUM.
  - Per-(head,chunk) ctx is evacuated to SBUF quickly to free PSUM, then
    normalized via a DRAM-bounce partition-broadcast of 1/l.
  - Out-projection consumes ctx^T directly; host sums partial outputs.
"""

from contextlib import ExitStack

import numpy as np
import ml_dtypes

import concourse.bass as bass
import concourse.tile as tile
from concourse import bacc, mybir
from concourse.bass_utils import run_bass_kernel_spmd

F32 = mybir.dt.float32
BF16 = mybir.dt.bfloat16
AF = mybir.ActivationFunctionType
P = 128
DD = 64
T = 2048
D = 1024
NH = 4            # heads per core
N_CORES = 8
EPS = 1e-6


def build_kernel(nc, reps=1):
    KT = D // P
    TT = T // P
    TC = T // 512
    NP = NH // 2
    QK = 2 * NH * DD
    VF = NH * DD

    xT_d = nc.dram_tensor("xT", [D, T], BF16, kind="ExternalInput")
    wqkT_d = nc.dram_tensor("wqkT", [D, QK], BF16, kind="ExternalInput")
    wvT_d = nc.dram_tensor("wvT", [D, VF], BF16, kind="ExternalInput")
    woT_d = nc.dram_tensor("woT", [VF, D], BF16, kind="ExternalInput")
    outT_d = nc.dram_tensor("outT", [D, T], F32, kind="ExternalOutput")

    with tile.TileContext(nc) as tc, ExitStack() as ctx:
        consts = ctx.enter_context(tc.tile_pool(name="consts", bufs=1))
        persist = ctx.enter_context(tc.tile_pool(name="persist", bufs=1))
        xsqp = ctx.enter_context(tc.tile_pool(name="xsqp", bufs=2))
        stmp = ctx.enter_context(tc.tile_pool(name="stmp", bufs=2))
        epool = ctx.enter_context(tc.tile_pool(name="epool", bufs=3))
        rlp = ctx.enter_context(tc.tile_pool(name="rlp", bufs=2))
        dramp = ctx.enter_context(tc.tile_pool(name="dramp", bufs=2, space="DRAM"))
        sps = ctx.enter_context(tc.tile_pool(name="sps", bufs=2, space="PSUM"))
        ps2 = ctx.enter_context(tc.tile_pool(name="ps2", bufs=4, space="PSUM"))

        def emit_body(iv=None):
            ones_bf = consts.tile([P, P], BF16)
            nc.vector.memset(ones_bf[:], 1.0)
            eps_sb = consts.tile([P, 1], F32)
            nc.vector.memset(eps_sb[:], EPS)
            mask_bf = consts.tile([P, P], BF16)
            nc.gpsimd.memset(mask_bf[:], 1.0)
            nc.gpsimd.affine_select(
                out=mask_bf[:], in_=mask_bf[:],
                compare_op=mybir.AluOpType.is_ge, fill=0.0, base=0,
                pattern=[[1, P]], channel_multiplier=-1,
            )

            # ---- x load (bf16) + RMSNorm -------------------------------
            xbf = persist.tile([P, KT, T], BF16)
            CPH = max(1, TC // 2)
            ms_h = []
            for half in range((TC + CPH - 1) // CPH):
                ms_half = sps.tile([P, 512 * CPH], F32, tag="big")
                ms_h.append(ms_half)
            for kt in range(KT):
                nc.sync.dma_start(xbf[:, kt, :], xT_d.ap()[kt * P : (kt + 1) * P, :])
                xsq = xsqp.tile([P, T], BF16)
                nc.vector.tensor_mul(xsq[:], xbf[:, kt, :], xbf[:, kt, :])
                for c in range(TC):
                    nc.tensor.matmul(
                        ms_h[c // CPH][:, 512 * (c % CPH) : 512 * (c % CPH) + 512],
                        ones_bf[:], xsq[:, 512 * c : 512 * (c + 1)],
                        start=(kt == 0), stop=(kt == KT - 1),
                    )
            rstd_bf = persist.tile([P, T], BF16)
            for c in range(TC):
                sq = stmp.tile([P, 512], F32, tag="sq")
                nc.scalar.activation(
                    sq[:], ms_h[c // CPH][:, 512 * (c % CPH) : 512 * (c % CPH) + 512],
                    AF.Sqrt, bias=eps_sb[:, 0:1], scale=1.0 / D,
                )
                with nc.allow_low_precision(reason="rstd feeds bf16 matmuls"):
                    nc.vector.reciprocal(rstd_bf[:, 512 * c : 512 * (c + 1)], sq[:])
            xn = persist.tile([P, KT, T], BF16)
            for kt in range(KT):
                nc.vector.tensor_mul(xn[:, kt, :], xbf[:, kt, :], rstd_bf[:])

            # ---- weights ------------------------------------------------
            wqk_bf = persist.tile([P, KT, QK], BF16)
            wv_bf = persist.tile([P, KT, VF], BF16)
            wo_bf = persist.tile([P, VF // P, D], BF16)
            for kt in range(KT):
                nc.sync.dma_start(wqk_bf[:, kt, :], wqkT_d.ap()[kt * P : (kt + 1) * P, :])
                nc.sync.dma_start(wv_bf[:, kt, :], wvT_d.ap()[kt * P : (kt + 1) * P, :])
            for ct in range(VF // P):
                nc.sync.dma_start(wo_bf[:, ct, :], woT_d.ap()[ct * P : (ct + 1) * P, :])

            QTd = persist.tile([P, NH, T], BF16)
            KTd = persist.tile([P, NH, T], BF16)
            Vsb = persist.tile([P, TT, 65 * NH], BF16)
            ctxn = persist.tile([P, NP, T], BF16)
            ctx_acc = []
            for _acc_i in range(2):
                acc_t = persist.tile([DD + 1, T], F32, tag=f"ctxacc{_acc_i}")
                ctx_acc.append(acc_t)

            def emit_qk_chunk(pair, is_k, c):
                ft = is_k * NP + pair
                dst = KTd if is_k else QTd
                hA, hB = 2 * pair, 2 * pair + 1
                qkps = ps2.tile([P, 512], F32, tag="mm")
                for kt in range(KT):
                    nc.tensor.matmul(
                        qkps[:],
                        wqk_bf[:, kt, P * ft : P * (ft + 1)],
                        xn[:, kt, 512 * c : 512 * (c + 1)],
                        start=(kt == 0), stop=(kt == KT - 1),
                    )
                cs = slice(512 * c, 512 * (c + 1))
                nc.vector.tensor_copy(dst[0:DD, hA, cs], qkps[0:DD, :])
                nc.vector.tensor_copy(dst[DD:P, hB, cs], qkps[DD:P, :])
                nc.sync.dma_start(dst[DD:P, hA, cs], dst[0:DD, hA, cs])
                nc.sync.dma_start(dst[0:DD, hB, cs], dst[DD:P, hB, cs])

            def emit_v_tiles(t0, t1):
                for tt in range(t0, t1):
                    vps = ps2.tile([P, 512], F32, tag="mm")
                    for kt in range(KT):
                        nc.tensor.matmul(
                            vps[:, :VF],
                            xn[:, kt, P * tt : P * (tt + 1)],
                            wv_bf[:, kt, :],
                            start=(kt == 0), stop=(kt == KT - 1),
                        )
                    for h in range(NH):
                        nc.vector.tensor_copy(
                            Vsb[:, tt, 65 * h : 65 * h + DD],
                            vps[:, DD * h : DD * (h + 1)],
                        )
                        nc.vector.memset(Vsb[:, tt, 65 * h + DD : 65 * h + 65], 1.0)

            def emit_attn_chunk(h, c):
                ctx_ps = ps2.tile([65, 512], F32, tag="mm")
                njt = 4 * (c + 1)
                for g in range(njt // 2):
                    sst = sps.tile([P, 1024], F32, tag="big")
                    for s in range(2):
                        j = 2 * g + s
                        rg = DD * (j % 2)
                        nc.tensor.matmul(
                            sst[:, 512 * s : 512 * (s + 1)],
                            KTd[rg : rg + DD, h, P * j : P * (j + 1)],
                            QTd[rg : rg + DD, h, 512 * c : 512 * (c + 1)],
                            start=True, stop=True,
                        )
                    expS = epool.tile([P, 1024], BF16)
                    nc.scalar.activation(expS[:], sst[:], AF.Exp, scale=0.125)
                    for s in range(2):
                        j = 2 * g + s
                        off = 128 * j - 512 * c
                        if 0 <= off < 512:
                            nc.vector.tensor_mul(
                                expS[:, 512 * s + off : 512 * s + off + P],
                                expS[:, 512 * s + off : 512 * s + off + P],
                                mask_bf[:],
                            )
                    for s in range(2):
                        j = 2 * g + s
                        off = max(0, 128 * j - 512 * c)
                        nc.tensor.matmul(
                            ctx_ps[:, off:512],
                            Vsb[:, j, 65 * h : 65 * (h + 1)],
                            expS[:, 512 * s + off : 512 * (s + 1)],
                            start=(j == 0), stop=(j == njt - 1),
                        )
                nc.vector.tensor_copy(
                    ctx_acc[h % 2][:, 512 * c : 512 * (c + 1)], ctx_ps[:, :]
                )

            def emit_norm(h, c):
                pair, half = h // 2, h % 2
                acc = ctx_acc[h % 2]
                cs = slice(512 * c, 512 * (c + 1))
                nc.vector.reciprocal(acc[DD : DD + 1, cs], acc[DD : DD + 1, cs])
                rl_dram = dramp.tile([1, 512], F32, tag="rld")
                nc.sync.dma_start(rl_dram[:], acc[DD : DD + 1, cs])
                rl = rlp.tile([DD, 512], F32, tag="rl")
                nc.sync.dma_start(rl[:], rl_dram[0:1, :].partition_broadcast(DD))
                if half == 0:
                    nc.vector.tensor_mul(ctxn[0:DD, pair, cs], acc[0:DD, cs], rl[:])
                else:
                    tmpb = rlp.tile([DD, 512], BF16, tag="tmpb")
                    nc.vector.tensor_mul(tmpb[:], acc[0:DD, cs], rl[:])
                    nc.sync.dma_start(ctxn[DD:P, pair, cs], tmpb[:])

            def emit_outproj(c):
                for e in range(D // P):
                    ops = ps2.tile([P, 512], F32, tag="mm")
                    for ct in range(NP):
                        nc.tensor.matmul(
                            ops[:],
                            wo_bf[:, ct, P * e : P * (e + 1)],
                            ctxn[:, ct, 512 * c : 512 * (c + 1)],
                            start=(ct == 0), stop=(ct == NP - 1),
                        )
                    osb = stmp.tile([P, 512], F32, tag="osb")
                    if e % 2 == 0:
                        nc.vector.tensor_copy(osb[:], ops[:])
                    else:
                        nc.scalar.copy(osb[:], ops[:])
                    nc.sync.dma_start(
                        outT_d.ap()[P * e : P * (e + 1), 512 * c : 512 * (c + 1)],
                        osb[:],
                    )

            for c in range(TC):
                emit_qk_chunk(0, 0, c)
                emit_qk_chunk(0, 1, c)
                emit_v_tiles(4 * c, 4 * (c + 1))
                emit_attn_chunk(0, c)
                emit_norm(0, c)
            for c in range(TC):
                emit_attn_chunk(1, c)
                emit_norm(1, c)
            for is_k in (0, 1):
                for c in range(TC):
                    emit_qk_chunk(1, is_k, c)
            for c in range(TC):
                emit_attn_chunk(2, c)
                emit_norm(2, c)
            for c in range(TC):
                emit_attn_chunk(3, c)
                emit_norm(3, c)
            for c in range(TC):
                emit_outproj(c)

        if reps == 1:
            emit_body()
        else:
            with tc.For_i(0, reps, 1) as iv:
                emit_body(iv)


_NC_CACHE = None


def _get_nc():
    global _NC_CACHE
    if _NC_CACHE is None:
        nc = bacc.Bacc(
            "TRN2", target_bir_lowering=False, debug=False, num_devices=N_CORES
        )
        build_kernel(nc)
        nc.compile()
        _NC_CACHE = nc
    return _NC_CACHE


def make_in_maps(x, norm_weight, qkv_w, out_w):
    x = np.asarray(x, dtype=np.float32)
    norm_weight = np.asarray(norm_weight, dtype=np.float32)
    qkv_w = np.asarray(qkv_w, dtype=np.float32)
    out_w = np.asarray(out_w, dtype=np.float32)
    # fold the RMSNorm weight into the projection weights (exact in fp32)
    qkv_eff = qkv_w * norm_weight[None, :]
    bf = ml_dtypes.bfloat16
    in_maps = []
    for core in range(N_CORES):
        b, hg = core // 4, core % 4
        r0 = 256 * hg
        xT = np.ascontiguousarray(x[b].T.astype(bf))
        wqkT = np.ascontiguousarray(
            np.concatenate(
                [qkv_eff[r0 : r0 + 256], qkv_eff[D + r0 : D + r0 + 256]], 0
            ).T.astype(bf)
        )
        wvT = np.ascontiguousarray(qkv_eff[2 * D + r0 : 2 * D + r0 + 256].T.astype(bf))
        woT = np.ascontiguousarray(out_w[:, r0 : r0 + 256].T.astype(bf))
        in_maps.append({"xT": xT, "wqkT": wqkT, "wvT": wvT, "woT": woT})
    return in_maps


def gather_output(results):
    out = np.empty((2, T, D), np.float32)
    for b in range(2):
        acc = results[4 * b]["outT"].astype(np.float32).copy()
        for hg in range(1, 4):
            acc += results[4 * b + hg]["outT"]
        out[b] = acc.T
    return out


def kernel(x, norm_weight, qkv_w, out_w):
    nc = _get_nc()
    in_maps = make_in_maps(x, norm_weight, qkv_w, out_w)
    res = run_bass_kernel_spmd(nc, in_maps, core_ids=list(range(N_CORES)))
    return gather_output(res.results)
